# revision 56
# baseline (speedup 1.0000x reference)
"""BertSelfAttention on 8 Trainium2 NeuronCores.

Sharding: 8 cores = 4 batches x 2 head-halves. Each core computes, for its
batch b and its 8 heads, the unnormalized attention output transposed
(out.T = V.T @ P.T per head) plus the softmax denominator row (via a ones
column appended to V). The host pre-transposes inputs (X.T, W.T slices,
cast to fp16) and does the final normalize/transpose/concat.

Single fused schedule (~338us HW vs ~369us for the two-phase version):
- The Q/K/V projections ride the attention k-loops as a micro-pipelined
  work queue (1-2 single matmuls pumped per k-step, EDF-ordered), hiding
  the ~82us of projection PE time under the exp stream.
- Scores for both heads of a pair land in one [128,1024] PSUM tile
  (parity-packed matmuls in disjoint PE row groups), so exp is one FD=1024
  ScalarE activation per k-step; the AV lags scores by TWO k-steps so the
  latency loop exp(k)->AV(k)->scores(k+2)->exp(k+2) stays off the cadence.
- A fraction of k-steps (4/16; 8/16 in the filler-free last pair) compute
  exp on the VectorE via a Schraudolph bitcast exp (fp32 tensor_scalar ->
  int16 out viewed as fp16), keeping ScalarE off the critical path.
  Softmax tolerates the ~3% Schraudolph error (num/den use the same
  approximation); measured output rel err 7e-3 vs 6e-4 all-exact.
- Prologue: dummy-matmul HAM warmup in the DMA shadow; DMA-friendly host
  layouts so prologue-critical transfers are contiguous; exp DMA queue
  (Scalar) carries only prologue-critical issues.

PSUM (16KB/partition): psA/psB [128,1024] f32 score double-buffer (8KB),
po0/po1 [65,512] AV accumulators (4KB), proj [128,512] x2 (4KB).
"""

import sys

if "/opt/trn_rl_repo" not in sys.path:
    sys.path.insert(0, "/opt/trn_rl_repo")

import numpy as np

import concourse.bass as bass  # noqa: F401  (registers bass machinery)
import concourse.tile as tile
from concourse import bacc, mybir
from concourse.bass_utils import run_bass_kernel_spmd

B, S, H = 4, 2048, 1024
NH, DH = 16, 64
NCORES = 8
HPC = 8            # heads per core
OC = HPC * DH      # 512 output features per core
HC = H // 128      # 8 contraction chunks of 128
DHE = DH + 1       # head dim + denominator column
NK = 16            # key tiles of 128
NQC = 4            # q chunks of 512

F16 = mybir.dt.float16
F32 = mybir.dt.float32
I16 = mybir.dt.int16
EXP = mybir.ActivationFunctionType.Exp
MUL = mybir.AluOpType.mult
ADD = mybir.AluOpType.add

# Schraudolph fp16 exp: exp(s/8) ~= bitcast16(round(A*s + B)).
# A = 0.125 * 2^10/ln2; B = 15*2^10 - 59.5 (mean-centering offset).
SCH_A = 0.125 * 1024.0 / float(np.log(2.0))
SCH_B = 15.0 * 1024.0 - 59.5
# k-steps whose exp runs on VectorE (Schraudolph) instead of ScalarE.
# Pairs 0-2 are PE-bound (projection fillers): 4/16 keeps ScalarE off the
# critical path. Pair 3 has no fillers and goes ScalarE-serial-bound at
# ~1167ns/step, so alternate engines there (8/16).
SCH_KS = ((2, 6, 10, 14), (0, 2, 4, 6, 8, 10, 12, 14))

_PROGRAM = None
LAST_RESULT = None  # BassKernelResults of the most recent kernel() call


def _emit_kernel(tc, out, xt, wqt, wkt, wvt):
    nc = tc.nc
    with (
        tc.tile_pool(name="persist", bufs=1) as persist,
        tc.tile_pool(name="ptp", bufs=6) as ptp,
        tc.tile_pool(name="ost", bufs=4) as ost,
        tc.tile_pool(name="psa", bufs=1, space="PSUM") as psa,
    ):
        xt_sb = persist.tile([128, HC, S], F16)
        wq_sb = persist.tile([128, 4, HC, 128], F16)
        wk_sb = persist.tile([128, 4, HC, 128], F16)
        wv_sb = persist.tile([128, HC, OC], F16)
        qt_sb = persist.tile([128, 4, S], F16)
        kt_sb = persist.tile([128, 4, S], F16)
        v_sb = persist.tile([128, NK, HPC * DHE], F16)

        # ~4us of dummy matmuls, first thing: HAM-warms the PE to 2.4GHz so
        # the prologue projections don't run at the cold 1.2GHz. Emitted
        # before the DMA section so nothing queues ahead of it; its memset
        # goes on the (otherwise idle) gpsimd engine queue.
        warm = persist.tile([128, 192], F16)
        nc.gpsimd.memset(warm[:], 0.125)
        # Target the ps0 PSUM tag (pool bufs=1 -> same buffer as the first
        # score tile, whose start=True clears it; in-order PE, no stall).
        wps = psa.tile([128, 1024], F32, tag="ps0", name="warmps")

        def dummy_mms(n):
            for _ in range(n):
                nc.tensor.matmul(
                    wps[:, 0:64], warm[:, 0:128], warm[:, 128:192],
                    start=True, stop=True,
                )

        dummy_mms(70)  # ~3.7us cold: crosses the HAM SHORT window

        # DMA layout. The Scalar engine also runs the exp stream, and DMA
        # issues can stall multi-us on ring credits -- so scalar carries ONLY
        # the few transfers that gate the prologue (then stays clean), sync
        # carries the early-need rest, and everything needed later than
        # ~25us goes to the gpsimd SWDGE queue (slow ~6us start, then fine).
        # The host pre-arranges weights/xt so every slice here is contiguous.
        nc.sync.dma_start(wk_sb[:, 0], wkt[:, 0])
        nc.scalar.dma_start(wq_sb[:, 0], wqt[:, 0])
        for hc in (0, 2, 4):  # xt quarter 0:512 gates q00/k00
            nc.sync.dma_start(xt_sb[:, hc, 0:512], xt[:, 0, hc, :])
        for hc in (1, 3, 5):
            nc.scalar.dma_start(xt_sb[:, hc, 0:512], xt[:, 0, hc, :])
        # gpsimd order tracks first-use: the last two prologue xt chunks
        # (the SWDGE queue starts ~10us in but beats the backlogged HW
        # queues), then wv/xt-q1 (pair0-qc0 V tiles and kt0-sc1)
        # interleaved, then xt q2/q3, then the remaining weights.
        for hc in (0, 1):  # wv0/1 first: they gate the prefetched V tiles
            nc.gpsimd.dma_start(wv_sb[:, hc, :], wvt[:, hc, :])
        for hc in (6, 7):
            nc.gpsimd.dma_start(xt_sb[:, hc, 0:512], xt[:, 0, hc, :])
        for hc in (2, 3, 4):
            nc.gpsimd.dma_start(wv_sb[:, hc, :], wvt[:, hc, :])
        for hc in range(HC):
            nc.gpsimd.dma_start(xt_sb[:, hc, 512:1024], xt[:, 1, hc, :])
        for hc in range(5, HC):
            nc.gpsimd.dma_start(wv_sb[:, hc, :], wvt[:, hc, :])
        for q in (2, 3):
            for hc in range(HC):
                nc.gpsimd.dma_start(
                    xt_sb[:, hc, q * 512 : (q + 1) * 512], xt[:, q, hc, :]
                )
        nc.gpsimd.dma_start(wk_sb[:, 1:4], wkt[:, 1:4])
        nc.gpsimd.dma_start(wq_sb[:, 1:4], wqt[:, 1:4])

        # set the per-head ones column (softmax denominator accumulator);
        # v_gen copies fill the data columns. Only the 128 ones-columns are
        # memset -- a full v_sb memset is ~7us of DVE that would queue ahead
        # of the prologue projection copies.
        nc.vector.memset(
            v_sb.rearrange("p k (h e) -> p k h e", e=DHE)[:, :, :, DH : DH + 1],
            1.0,
        )

        def proj_gen(w_sb, c, sc, dst):
            p = psa.tile([128, 512], F32, tag="pj", bufs=2, name="pj")
            for hc in range(HC):
                if hc:
                    yield
                nc.tensor.matmul(
                    p[:],
                    w_sb[:, c, hc, :],
                    xt_sb[:, hc, sc * 512 : (sc + 1) * 512],
                    start=(hc == 0),
                    stop=(hc == HC - 1),
                )
            nc.vector.tensor_copy(dst[:, c, sc * 512 : (sc + 1) * 512], p[:])

        def v_gen(st):
            p = psa.tile([128, 512], F32, tag="pj", bufs=2, name="pjv")
            for hc in range(HC):
                if hc:
                    yield
                nc.tensor.matmul(
                    p[:],
                    xt_sb[:, hc, st * 128 : (st + 1) * 128],
                    wv_sb[:, hc, :],
                    start=(hc == 0),
                    stop=(hc == HC - 1),
                )
            nc.vector.tensor_copy(
                v_sb[:, st, :].rearrange("p (h e) -> p h e", e=DHE)[:, :, 0:DH],
                p[:].rearrange("p (h d) -> p h d", d=DH),
            )

        # Projections ride the attention k-loops as a micro-pipelined work
        # queue: each k-step pumps a few single matmuls (not whole 8-matmul
        # tiles) into the PE stream right where the AV would otherwise sit
        # waiting on the exp semaphore. Queue order is earliest-deadline.
        from collections import deque

        feed = deque()

        def pump(n):
            # one unit == one projection matmul (exhaustion carries the
            # tile's last matmul + its PSUM->SBUF copy)
            while n > 0 and feed:
                try:
                    next(feed[0])
                except StopIteration:
                    feed.popleft()
                n -= 1

        def q_tile(c, sc):
            return proj_gen(wq_sb, c, sc, qt_sb)

        def k_tile(c, sc):
            return proj_gen(wk_sb, c, sc, kt_sb)

        def qk_tiles(c):
            return [proj_gen(w, c, sc, d)
                    for w, d in ((wq_sb, qt_sb), (wk_sb, kt_sb))
                    for sc in range(4)]

        # ---- prologue: the sc0 projections of chunk 0 plus the first three
        # V tiles (their wv/xt DMAs land mid-prologue; this thins the
        # overloaded pair0-qc0 filler zone). The matmuls are DMA-paced
        # (sparse) -- interleave dummy matmuls so HAM doesn't re-throttle
        # the PE mid-prologue.
        v = [v_gen(st) for st in range(NK)]
        for g in (q_tile(0, 0), k_tile(0, 0), v[0], v[1], v[2], v[3]):
            feed.append(g)
        for _ in range(48):
            pump(1)
            dummy_mms(1)

        # ---- fused attention + projection schedule ----
        # Deadlines: kt-sc j of chunk c is first read at (pair c, qc0, k=4j);
        # qt-sc j at (pair c, qc j, k=0); v(st) at AV(pair0, qc0, k=st).
        # Each pair's k-loops carry its own late tiles (kt-sc2/3, qt-sc1..3)
        # plus the next chunk's early tiles (qt-sc0, kt-sc0/1), EDF-ordered,
        # so the pump load stays near 1 matmul/step outside pair0-qc0.
        qt = [[q_tile(c, sc) for sc in range(4)] for c in range(4)]
        kt = [[k_tile(c, sc) for sc in range(4)] for c in range(4)]
        feeds = {
            # q3 zones are chain-bound with ~3us of PE slack each: park one
            # next-chunk tile there instead of fattening the rate-2 q0 zones.
            0: {
                0: ([v[4], v[5], kt[0][1], v[6], v[7], kt[0][2],
                     v[8], v[9], v[10], kt[0][3],
                     v[11], v[12], v[13], v[14], qt[0][1], v[15]], 8),
                1: ([qt[0][2]], 1),
                2: ([qt[0][3], kt[1][0]], 1),
                3: ([kt[1][1], qt[1][0]], 1),
            },
            1: {
                0: ([kt[1][2], kt[1][3], qt[1][1]], 2),
                1: ([qt[1][2], kt[2][0]], 1),
                2: ([qt[1][3], kt[2][1]], 1),
                3: ([qt[2][0]], 1),
            },
            2: {
                0: ([kt[2][2], kt[2][3], qt[2][1]], 2),
                1: ([qt[2][2], kt[3][0]], 1),
                2: ([qt[2][3], kt[3][1]], 1),
                3: ([qt[3][0]], 1),
            },
            3: {
                0: ([kt[3][2], kt[3][3], qt[3][1]], 2),
                1: ([qt[3][2]], 1),
                2: ([qt[3][3]], 1),
                3: ([], 0),
            },
        }

        for pair in range(4):
            for qc in range(NQC):
                qsl = slice(qc * 512, (qc + 1) * 512)
                po = [psa.tile([DHE, 512], F32, tag=f"po{p}", name=f"po{p}")
                      for p in range(2)]
                gens, rate = feeds[pair].get(qc, ([], 0))
                # chain-bound zones (pair 3 and any light-feed qc) alternate
                # exp engines per step: consecutive same-engine exps serialize
                # the exp->scores->exp chain at ~1114ns/step otherwise.
                sch = SCH_KS[1] if (pair == 3 or len(gens) <= 1) else SCH_KS[0]
                feed.extend(gens)
                pending = []  # (k, pt): AV lags scores/exp by one k step

                def av(k, pt, mid=None):
                    for p in range(2):
                        if p and mid:
                            mid()  # pump slot between the AVs hides V1's LDW
                        hsl = slice((2 * pair + p) * DHE,
                                    (2 * pair + p + 1) * DHE)
                        nc.tensor.matmul(
                            po[p][:],
                            v_sb[:, k, hsl],
                            pt[:, p * 512 : (p + 1) * 512],
                            start=(k == 0),
                            stop=(k == NK - 1),
                        )

                for k in range(NK):
                    ksl = slice(k * 128, (k + 1) * 128)
                    ps = psa.tile([128, 1024], F32, tag=f"ps{k % 2}",
                                  name=f"ps{k % 2}")
                    pt = ptp.tile([128, 1024], F16, tag="pt", name="pt")
                    for p in range(2):  # head parity: PE rows 0-63 / 64-127
                        rows = slice(p * 64, (p + 1) * 64)
                        nc.tensor.matmul(
                            ps[:, p * 512 : (p + 1) * 512],
                            kt_sb[rows, pair, ksl],
                            qt_sb[rows, pair, qsl],
                            start=True,
                            stop=True,
                        )
                    if rate > 1:
                        pump(rate - 1)
                    if k in sch:
                        nc.vector.tensor_scalar(
                            pt.bitcast(I16), ps[:], SCH_A, SCH_B, MUL, ADD
                        )
                    else:
                        nc.scalar.activation(pt[:], ps[:], EXP, scale=0.125)
                    pending.append((k, pt))
                    # AV lags by TWO steps: scores(k+2) must issue on the PE
                    # before AV(k), else the latency loop exp(k) -> AV(k) ->
                    # scores(k+2) -> exp(k+2) sets a ~1130ns/step cadence.
                    if len(pending) > 2:
                        av(*pending.pop(0),
                           mid=(lambda: pump(1)) if rate else None)
                    elif rate:
                        pump(1)
                for item in pending:
                    av(*item)
                for p in range(2):
                    o = ost.tile([DHE, 512], F32, tag="o", name="o")
                    nc.vector.tensor_copy(o[:], po[p][:])
                    nc.sync.dma_start(out[2 * pair + p, :, qsl], o[:])


def _get_program():
    global _PROGRAM
    if _PROGRAM is None:
        nc = bacc.Bacc(
            "TRN2", target_bir_lowering=False, debug=False, num_devices=NCORES
        )
        xt = nc.dram_tensor("xt", [128, 4, HC, 512], F16, kind="ExternalInput").ap()
        wqt = nc.dram_tensor("wqt", [128, 4, HC, 128], F16, kind="ExternalInput").ap()
        wkt = nc.dram_tensor("wkt", [128, 4, HC, 128], F16, kind="ExternalInput").ap()
        wvt = nc.dram_tensor("wvt", [128, HC, OC], F16, kind="ExternalInput").ap()
        out = nc.dram_tensor("out", [HPC, DHE, S], F32, kind="ExternalOutput").ap()
        with tile.TileContext(nc) as tc:
            _emit_kernel(tc, out, xt, wqt, wkt, wvt)
        nc.compile()
        _PROGRAM = nc
    return _PROGRAM


def kernel(**inputs):
    global LAST_RESULT
    X = np.asarray(inputs["hidden_states"], dtype=np.float32)
    Ws = {k: np.asarray(inputs[k], dtype=np.float32) for k in ("Wq", "Wk", "Wv")}

    nc = _get_program()

    # DMA-friendly host layouts: xt [p, q-quarter, hc, 512], w [p, c, hc, 128]
    # (wv [p, hc, 512]) -- every device-side DMA slice is then contiguous.
    def _xt(b):
        a = X[b].T.astype(np.float16)              # [H, S]
        return np.ascontiguousarray(
            a.reshape(HC, 128, 4, 512).transpose(1, 2, 0, 3)
        )

    def _w(W, sl):
        a = W[sl].T.astype(np.float16)             # [H, OC]
        return np.ascontiguousarray(
            a.reshape(HC, 128, 4, 128).transpose(1, 2, 0, 3)
        )

    def _wv(W, sl):
        a = W[sl].T.astype(np.float16)
        return np.ascontiguousarray(a.reshape(HC, 128, OC).transpose(1, 0, 2))

    in_maps = []
    for core in range(NCORES):
        b, half = core // 2, core % 2
        sl = slice(half * OC, (half + 1) * OC)
        in_maps.append(
            {
                "xt": _xt(b),
                "wqt": _w(Ws["Wq"], sl),
                "wkt": _w(Ws["Wk"], sl),
                "wvt": _wv(Ws["Wv"], sl),
            }
        )

    LAST_RESULT = run_bass_kernel_spmd(nc, in_maps, core_ids=list(range(NCORES)))

    out = np.empty((B, S, H), dtype=np.float32)
    for core in range(NCORES):
        r = LAST_RESULT.results[core]["out"]          # [HPC, DHE, S]
        num = r[:, :DH, :]                            # [8, 64, 2048]
        den = r[:, DH : DH + 1, :]                    # [8, 1, 2048]
        o = (num / den).transpose(2, 0, 1).reshape(S, OC)
        b, half = core // 2, core % 2
        out[b, :, half * OC : (half + 1) * OC] = o
    return out


# revision 58
# speedup vs baseline: 1.0060x; 1.0060x over previous
"""BertSelfAttention on 8 Trainium2 NeuronCores.

Sharding: 8 cores = 4 batches x 2 head-halves. Each core computes, for its
batch b and its 8 heads, the unnormalized attention output transposed
(out.T = V.T @ P.T per head) plus the softmax denominator row (via a ones
column appended to V). The host pre-transposes inputs (X.T, W.T slices,
cast to fp16) and does the final normalize/transpose/concat.

Single fused schedule (~338us HW vs ~369us for the two-phase version):
- The Q/K/V projections ride the attention k-loops as a micro-pipelined
  work queue (1-2 single matmuls pumped per k-step, EDF-ordered), hiding
  the ~82us of projection PE time under the exp stream.
- Scores for both heads of a pair land in one [128,1024] PSUM tile
  (parity-packed matmuls in disjoint PE row groups), so exp is one FD=1024
  ScalarE activation per k-step; the AV lags scores by TWO k-steps so the
  latency loop exp(k)->AV(k)->scores(k+2)->exp(k+2) stays off the cadence.
- A fraction of k-steps (4/16; 8/16 in the filler-free last pair) compute
  exp on the VectorE via a Schraudolph bitcast exp (fp32 tensor_scalar ->
  int16 out viewed as fp16), keeping ScalarE off the critical path.
  Softmax tolerates the ~3% Schraudolph error (num/den use the same
  approximation); measured output rel err 7e-3 vs 6e-4 all-exact.
- Prologue: dummy-matmul HAM warmup in the DMA shadow; DMA-friendly host
  layouts so prologue-critical transfers are contiguous; exp DMA queue
  (Scalar) carries only prologue-critical issues.

PSUM (16KB/partition): psA/psB [128,1024] f32 score double-buffer (8KB),
po0/po1 [65,512] AV accumulators (4KB), proj [128,512] x2 (4KB).
"""

import sys

if "/opt/trn_rl_repo" not in sys.path:
    sys.path.insert(0, "/opt/trn_rl_repo")

import numpy as np

import concourse.bass as bass  # noqa: F401  (registers bass machinery)
import concourse.tile as tile
from concourse import bacc, mybir
from concourse.bass_utils import run_bass_kernel_spmd

B, S, H = 4, 2048, 1024
NH, DH = 16, 64
NCORES = 8
HPC = 8            # heads per core
OC = HPC * DH      # 512 output features per core
HC = H // 128      # 8 contraction chunks of 128
DHE = DH + 1       # head dim + denominator column
NK = 16            # key tiles of 128
NQC = 4            # q chunks of 512

F16 = mybir.dt.float16
F32 = mybir.dt.float32
I16 = mybir.dt.int16
EXP = mybir.ActivationFunctionType.Exp
MUL = mybir.AluOpType.mult
ADD = mybir.AluOpType.add

# Schraudolph fp16 exp: exp(s/8) ~= bitcast16(round(A*s + B)).
# A = 0.125 * 2^10/ln2; B = 15*2^10 - 59.5 (mean-centering offset).
SCH_A = 0.125 * 1024.0 / float(np.log(2.0))
SCH_B = 15.0 * 1024.0 - 59.5
# k-steps whose exp runs on VectorE (Schraudolph) instead of ScalarE.
# Pairs 0-2 are PE-bound (projection fillers): 4/16 keeps ScalarE off the
# critical path. Pair 3 has no fillers and goes ScalarE-serial-bound at
# ~1167ns/step, so alternate engines there (8/16).
SCH_KS = ((2, 6, 10, 14), (0, 2, 4, 6, 8, 10, 12, 14))

_PROGRAM = None
LAST_RESULT = None  # BassKernelResults of the most recent kernel() call


def _emit_kernel(tc, out, xt, wqt, wkt, wvt):
    nc = tc.nc
    with (
        tc.tile_pool(name="persist", bufs=1) as persist,
        tc.tile_pool(name="ptp", bufs=6) as ptp,
        tc.tile_pool(name="ost", bufs=4) as ost,
        tc.tile_pool(name="psa", bufs=1, space="PSUM") as psa,
    ):
        xt_sb = persist.tile([128, HC, S], F16)
        wq_sb = persist.tile([128, 4, HC, 128], F16)
        wk_sb = persist.tile([128, 4, HC, 128], F16)
        wv_sb = persist.tile([128, HC, OC], F16)
        qt_sb = persist.tile([128, 4, S], F16)
        kt_sb = persist.tile([128, 4, S], F16)
        v_sb = persist.tile([128, NK, HPC * DHE], F16)

        # ~4us of dummy matmuls, first thing: HAM-warms the PE to 2.4GHz so
        # the prologue projections don't run at the cold 1.2GHz. Emitted
        # before the DMA section so nothing queues ahead of it; its memset
        # goes on the (otherwise idle) gpsimd engine queue.
        warm = persist.tile([128, 192], F16)
        nc.gpsimd.memset(warm[:], 0.125)
        # Target the ps0 PSUM tag (pool bufs=1 -> same buffer as the first
        # score tile, whose start=True clears it; in-order PE, no stall).
        wps = psa.tile([128, 1024], F32, tag="ps0", name="warmps")

        def dummy_mms(n):
            for _ in range(n):
                nc.tensor.matmul(
                    wps[:, 0:64], warm[:, 0:128], warm[:, 128:192],
                    start=True, stop=True,
                )

        dummy_mms(70)  # ~3.7us cold: crosses the HAM SHORT window

        # DMA layout. The Scalar engine also runs the exp stream, and DMA
        # issues can stall multi-us on ring credits -- so scalar carries ONLY
        # the few transfers that gate the prologue (then stays clean), sync
        # carries the early-need rest, and everything needed later than
        # ~25us goes to the gpsimd SWDGE queue (slow ~6us start, then fine).
        # The host pre-arranges weights/xt so every slice here is contiguous.
        nc.sync.dma_start(wk_sb[:, 0], wkt[:, 0])
        nc.scalar.dma_start(wq_sb[:, 0], wqt[:, 0])
        for hc in (0, 2, 4):  # xt quarter 0:512 gates q00/k00
            nc.sync.dma_start(xt_sb[:, hc, 0:512], xt[:, 0, hc, :])
        for hc in (1, 3, 5):
            nc.scalar.dma_start(xt_sb[:, hc, 0:512], xt[:, 0, hc, :])
        # gpsimd order tracks first-use: the last two prologue xt chunks
        # (the SWDGE queue starts ~10us in but beats the backlogged HW
        # queues), then wv/xt-q1 (pair0-qc0 V tiles and kt0-sc1)
        # interleaved, then xt q2/q3, then the remaining weights.
        for hc in (6, 7):
            nc.gpsimd.dma_start(xt_sb[:, hc, 0:512], xt[:, 0, hc, :])
        for hc in range(3):
            nc.gpsimd.dma_start(wv_sb[:, hc, :], wvt[:, hc, :])
        for hc in range(HC):
            nc.gpsimd.dma_start(xt_sb[:, hc, 512:1024], xt[:, 1, hc, :])
        for hc in range(3, HC):
            nc.gpsimd.dma_start(wv_sb[:, hc, :], wvt[:, hc, :])
        for q in (2, 3):
            for hc in range(HC):
                nc.gpsimd.dma_start(
                    xt_sb[:, hc, q * 512 : (q + 1) * 512], xt[:, q, hc, :]
                )
        nc.gpsimd.dma_start(wk_sb[:, 1:4], wkt[:, 1:4])
        nc.gpsimd.dma_start(wq_sb[:, 1:4], wqt[:, 1:4])

        # set the per-head ones column (softmax denominator accumulator);
        # v_gen copies fill the data columns. Only the 128 ones-columns are
        # memset -- a full v_sb memset is ~7us of DVE that would queue ahead
        # of the prologue projection copies.
        nc.vector.memset(
            v_sb.rearrange("p k (h e) -> p k h e", e=DHE)[:, :, :, DH : DH + 1],
            1.0,
        )

        def proj_gen(w_sb, c, sc, dst):
            p = psa.tile([128, 512], F32, tag="pj", bufs=2, name="pj")
            for hc in range(HC):
                if hc:
                    yield
                nc.tensor.matmul(
                    p[:],
                    w_sb[:, c, hc, :],
                    xt_sb[:, hc, sc * 512 : (sc + 1) * 512],
                    start=(hc == 0),
                    stop=(hc == HC - 1),
                )
            nc.vector.tensor_copy(dst[:, c, sc * 512 : (sc + 1) * 512], p[:])

        def v_gen(st):
            p = psa.tile([128, 512], F32, tag="pj", bufs=2, name="pjv")
            for hc in range(HC):
                if hc:
                    yield
                nc.tensor.matmul(
                    p[:],
                    xt_sb[:, hc, st * 128 : (st + 1) * 128],
                    wv_sb[:, hc, :],
                    start=(hc == 0),
                    stop=(hc == HC - 1),
                )
            nc.vector.tensor_copy(
                v_sb[:, st, :].rearrange("p (h e) -> p h e", e=DHE)[:, :, 0:DH],
                p[:].rearrange("p (h d) -> p h d", d=DH),
            )

        # Projections ride the attention k-loops as a micro-pipelined work
        # queue: each k-step pumps a few single matmuls (not whole 8-matmul
        # tiles) into the PE stream right where the AV would otherwise sit
        # waiting on the exp semaphore. Queue order is earliest-deadline.
        from collections import deque

        feed = deque()

        def pump(n):
            # one unit == one projection matmul (exhaustion carries the
            # tile's last matmul + its PSUM->SBUF copy)
            while n > 0 and feed:
                try:
                    next(feed[0])
                except StopIteration:
                    feed.popleft()
                n -= 1

        def q_tile(c, sc):
            return proj_gen(wq_sb, c, sc, qt_sb)

        def k_tile(c, sc):
            return proj_gen(wk_sb, c, sc, kt_sb)

        def qk_tiles(c):
            return [proj_gen(w, c, sc, d)
                    for w, d in ((wq_sb, qt_sb), (wk_sb, kt_sb))
                    for sc in range(4)]

        # ---- prologue: the sc0 projections of chunk 0 plus the first three
        # V tiles (their wv/xt DMAs land mid-prologue; this thins the
        # overloaded pair0-qc0 filler zone). The matmuls are DMA-paced
        # (sparse) -- interleave dummy matmuls so HAM doesn't re-throttle
        # the PE mid-prologue.
        v = [v_gen(st) for st in range(NK)]
        for g in (q_tile(0, 0), k_tile(0, 0), v[0], v[1], v[2]):
            feed.append(g)
        for _ in range(40):
            pump(1)
            dummy_mms(1)

        # ---- fused attention + projection schedule ----
        # Deadlines: kt-sc j of chunk c is first read at (pair c, qc0, k=4j);
        # qt-sc j at (pair c, qc j, k=0); v(st) at AV(pair0, qc0, k=st).
        # Each pair's k-loops carry its own late tiles (kt-sc2/3, qt-sc1..3)
        # plus the next chunk's early tiles (qt-sc0, kt-sc0/1), EDF-ordered,
        # so the pump load stays near 1 matmul/step outside pair0-qc0.
        qt = [[q_tile(c, sc) for sc in range(4)] for c in range(4)]
        kt = [[k_tile(c, sc) for sc in range(4)] for c in range(4)]
        feeds = {
            # q3 zones are chain-bound with ~3us of PE slack each: park one
            # next-chunk tile there instead of fattening the rate-2 q0 zones.
            0: {
                0: ([v[3], kt[0][1], v[4], v[5], v[6], kt[0][2],
                     v[7], v[8], v[9], v[10], kt[0][3],
                     v[11], v[12], v[13], v[14], qt[0][1], v[15]], 9),
                1: ([qt[0][2]], 1),
                2: ([qt[0][3], kt[1][0]], 1),
                3: ([kt[1][1], qt[1][0]], 1),
            },
            1: {
                0: ([kt[1][2], kt[1][3], qt[1][1]], 2),
                1: ([qt[1][2], kt[2][0]], 1),
                2: ([qt[1][3], kt[2][1]], 1),
                3: ([qt[2][0]], 1),
            },
            2: {
                0: ([kt[2][2], kt[2][3], qt[2][1]], 2),
                1: ([qt[2][2], kt[3][0]], 1),
                2: ([qt[2][3], kt[3][1]], 1),
                3: ([qt[3][0]], 1),
            },
            3: {
                0: ([kt[3][2], kt[3][3], qt[3][1]], 2),
                1: ([qt[3][2]], 1),
                2: ([qt[3][3]], 1),
                3: ([], 0),
            },
        }

        for pair in range(4):
            sch = SCH_KS[1] if pair == 3 else SCH_KS[0]
            for qc in range(NQC):
                qsl = slice(qc * 512, (qc + 1) * 512)
                po = [psa.tile([DHE, 512], F32, tag=f"po{p}", name=f"po{p}")
                      for p in range(2)]
                gens, rate = feeds[pair].get(qc, ([], 0))
                feed.extend(gens)
                pending = []  # (k, pt): AV lags scores/exp by one k step

                def av(k, pt, mid=None):
                    for p in range(2):
                        if p and mid:
                            mid()  # pump slot between the AVs hides V1's LDW
                        hsl = slice((2 * pair + p) * DHE,
                                    (2 * pair + p + 1) * DHE)
                        nc.tensor.matmul(
                            po[p][:],
                            v_sb[:, k, hsl],
                            pt[:, p * 512 : (p + 1) * 512],
                            start=(k == 0),
                            stop=(k == NK - 1),
                        )

                for k in range(NK):
                    ksl = slice(k * 128, (k + 1) * 128)
                    ps = psa.tile([128, 1024], F32, tag=f"ps{k % 2}",
                                  name=f"ps{k % 2}")
                    pt = ptp.tile([128, 1024], F16, tag="pt", name="pt")
                    for p in range(2):  # head parity: PE rows 0-63 / 64-127
                        rows = slice(p * 64, (p + 1) * 64)
                        nc.tensor.matmul(
                            ps[:, p * 512 : (p + 1) * 512],
                            kt_sb[rows, pair, ksl],
                            qt_sb[rows, pair, qsl],
                            start=True,
                            stop=True,
                        )
                    if rate > 1:
                        pump(rate - 1)
                    if k in sch:
                        nc.vector.tensor_scalar(
                            pt.bitcast(I16), ps[:], SCH_A, SCH_B, MUL, ADD
                        )
                    else:
                        nc.scalar.activation(pt[:], ps[:], EXP, scale=0.125)
                    pending.append((k, pt))
                    # AV lags by TWO steps: scores(k+2) must issue on the PE
                    # before AV(k), else the latency loop exp(k) -> AV(k) ->
                    # scores(k+2) -> exp(k+2) sets a ~1130ns/step cadence.
                    if len(pending) > 2:
                        av(*pending.pop(0),
                           mid=(lambda: pump(1)) if rate else None)
                    elif rate:
                        pump(1)
                for item in pending:
                    av(*item)
                for p in range(2):
                    o = ost.tile([DHE, 512], F32, tag="o", name="o")
                    nc.vector.tensor_copy(o[:], po[p][:])
                    nc.sync.dma_start(out[2 * pair + p, :, qsl], o[:])


def _get_program():
    global _PROGRAM
    if _PROGRAM is None:
        nc = bacc.Bacc(
            "TRN2", target_bir_lowering=False, debug=False, num_devices=NCORES
        )
        xt = nc.dram_tensor("xt", [128, 4, HC, 512], F16, kind="ExternalInput").ap()
        wqt = nc.dram_tensor("wqt", [128, 4, HC, 128], F16, kind="ExternalInput").ap()
        wkt = nc.dram_tensor("wkt", [128, 4, HC, 128], F16, kind="ExternalInput").ap()
        wvt = nc.dram_tensor("wvt", [128, HC, OC], F16, kind="ExternalInput").ap()
        out = nc.dram_tensor("out", [HPC, DHE, S], F32, kind="ExternalOutput").ap()
        with tile.TileContext(nc) as tc:
            _emit_kernel(tc, out, xt, wqt, wkt, wvt)
        nc.compile()
        _PROGRAM = nc
    return _PROGRAM


def kernel(**inputs):
    global LAST_RESULT
    X = np.asarray(inputs["hidden_states"], dtype=np.float32)
    Ws = {k: np.asarray(inputs[k], dtype=np.float32) for k in ("Wq", "Wk", "Wv")}

    nc = _get_program()

    # DMA-friendly host layouts: xt [p, q-quarter, hc, 512], w [p, c, hc, 128]
    # (wv [p, hc, 512]) -- every device-side DMA slice is then contiguous.
    def _xt(b):
        a = X[b].T.astype(np.float16)              # [H, S]
        return np.ascontiguousarray(
            a.reshape(HC, 128, 4, 512).transpose(1, 2, 0, 3)
        )

    def _w(W, sl):
        a = W[sl].T.astype(np.float16)             # [H, OC]
        return np.ascontiguousarray(
            a.reshape(HC, 128, 4, 128).transpose(1, 2, 0, 3)
        )

    def _wv(W, sl):
        a = W[sl].T.astype(np.float16)
        return np.ascontiguousarray(a.reshape(HC, 128, OC).transpose(1, 0, 2))

    in_maps = []
    for core in range(NCORES):
        b, half = core // 2, core % 2
        sl = slice(half * OC, (half + 1) * OC)
        in_maps.append(
            {
                "xt": _xt(b),
                "wqt": _w(Ws["Wq"], sl),
                "wkt": _w(Ws["Wk"], sl),
                "wvt": _wv(Ws["Wv"], sl),
            }
        )

    LAST_RESULT = run_bass_kernel_spmd(nc, in_maps, core_ids=list(range(NCORES)))

    out = np.empty((B, S, H), dtype=np.float32)
    for core in range(NCORES):
        r = LAST_RESULT.results[core]["out"]          # [HPC, DHE, S]
        num = r[:, :DH, :]                            # [8, 64, 2048]
        den = r[:, DH : DH + 1, :]                    # [8, 1, 2048]
        o = (num / den).transpose(2, 0, 1).reshape(S, OC)
        b, half = core // 2, core % 2
        out[b, :, half * OC : (half + 1) * OC] = o
    return out
